# revision 12
# baseline (speedup 1.0000x reference)
"""DeepseekMoE kernel for 8 Trainium2 NeuronCores.

Strategy (expert-parallel routed + data-parallel shared, fp8 PE path):
  - Host computes the router (gate matmul, softmax, top-2) in numpy and
    gathers each expert's tokens (classic MoE dispatch, done host-side as
    part of sharding).  Core c runs routed expert c's FFN over its
    gathered tokens (padded to a common cpad so all 8 cores run the same
    SPMD program); shared experts are replicated and each core runs them
    over a distinct 512-token slice of the batch.
  - All matmuls run in fp8-E4M3 with the PE's DoubleRow perf mode (two
    128-row k-tiles contracted per instruction at 0.5 cycles/row -> 4x
    the fp16 rate).  Weights are scaled by 64 before quantization so
    their 0.02-scale values leave the E4M3 subnormal range; the 1/64
    descale folds into the GELU's input scale (layer 1), the combine
    weights (routed layer 2) or the output copy (shared layer 2).
  - The routed path (only ~4.5% of the output energy after the top-2
    combine weights) tolerates plain fp8 quantization of x, h and both
    weight matrices (~1.1e-2 end-to-end rel err vs the 2e-2 budget).
  - The shared path (~97% of output energy) cannot; each shared operand
    is a same-scale fp8 PAIR (hi = fp8(v), lo = fp8(v - hi)) and each
    matmul contracts 3 of the 4 cross terms -- hi@a + lo@a + hi@b --
    which restores ~fp16 accuracy at 0.75x the fp16 PE cost.  x's pair
    is built on the host; h's pair is built on-device (ACT writes
    gelu->fp16, DVE casts the fp8 hi image, Pool subtracts for lo).
  - Shared phase runs FIRST: its 8 MB of weight DMA streams in
    consumption order behind a 41 us compute phase, and the (smaller)
    routed weights stream during the shared tail, so the DMA engines are
    never on the critical path.
  - Layout is fully transposed (features on partitions, tokens on the
    free dim) so the two FFN layers chain with no on-chip transposes;
    every operand is host-packed so each DMA is a contiguous >=2KB-per-
    partition transfer.  Outputs ride the SWDGE (gpsimd) path except the
    final chunk (HWDGE, shorter tail).
  - Host scatters per-expert outputs back (each token appears in K=2
    experts) and adds the (zero, but handled exactly) output biases.
"""

import numpy as np
import ml_dtypes

import concourse.bass as bass
import concourse.tile as tile
import concourse.mybir as mybir
from concourse import bacc
from concourse.bass_utils import run_bass_kernel_spmd

B, S, D, F, E, NS, K = 2, 2048, 512, 2048, 8, 2, 2
T = B * S
N_CORES = 8
TS = T // N_CORES          # shared-expert tokens per core
FS = NS * F                # concatenated shared FFN width (4096)
CHUNK = 512                # token chunk (= max fp32 PSUM bank free dim)
KD = D // 128              # 4  k-tiles over D
FR = F // 128              # 16 f-tiles routed
FPR = FR // 2              # 8  routed f-pairs
FSH = FS // 128            # 32 f-tiles shared
FPS = FSH // 2             # 16 shared f-pairs
DD = D // 128              # 4  output d-tiles
SC = 64.0                  # fp8 weight pre-scale (lifts 0.02-scale
                           # weights out of E4M3's subnormal range)

F8 = mybir.dt.float8e4
F16 = mybir.dt.float16
BF16 = mybir.dt.bfloat16
F32 = mybir.dt.float32
np_f8 = ml_dtypes.float8_e4m3
np_bf16 = ml_dtypes.bfloat16

DR = mybir.MatmulPerfMode.DoubleRow
_GELU = mybir.ActivationFunctionType.Gelu

_cache: dict = {}


def _routed_sizes(cpad):
    """Token-chunk sizes for the routed phase: a mid-size first chunk,
    512s in the middle, and a small final chunk (short drain tail)."""
    if cpad <= CHUNK:
        return [cpad]
    head = cpad - 768
    if 256 <= head <= CHUNK:
        return [head, CHUNK, 256]
    if cpad < 1024:
        mid = cpad - 512
        return [256] + ([mid] if mid else []) + [256]
    sizes, rem = [256], cpad - 768
    while rem > CHUNK:
        take = CHUNK if rem - CHUNK >= 256 else rem - 256
        sizes.append(take)
        rem -= take
    sizes.append(rem)
    return sizes + [256, 256]


def _chunk_offsets(total, sizes):
    out, c0 = [], 0
    for s in sizes:
        out.append((c0, s))
        c0 += s
    return out


def _build(cpad: int, biased: bool = False):
    nc = bacc.Bacc("TRN2", debug=False)

    # -- routed inputs (expert c on core c), all fp8 pre-scaled by 64 --
    xg = nc.dram_tensor("xg", [128, KD * cpad], F8, kind="ExternalInput")
    cwb = nc.dram_tensor("cwb", [128, cpad], F16, kind="ExternalInput")
    # rw1: col = f*(KD*128) + k*128 ; stationary [k-rows, f-cols] blocks
    rw1t = nc.dram_tensor("rw1t", [128, FR * KD * 128], F8, kind="ExternalInput")
    # rw2: col = fp*(DD*256) + d*256 + two*128
    rw2t = nc.dram_tensor("rw2t", [128, FPR * DD * 256], F8, kind="ExternalInput")
    rb1 = nc.dram_tensor("rb1", [128, FR], F32, kind="ExternalInput")
    # -- shared inputs, fp8 hi/lo pairs --
    # xs: col = img*(KD*TS) + k*TS   (img 0 = hi, 1 = lo)
    xs = nc.dram_tensor("xs", [128, 2 * KD * TS], F8, kind="ExternalInput")
    # sw1: col = f*(2*KD*128) + img*(KD*128) + k*128
    sw1t = nc.dram_tensor("sw1t", [128, FSH * 2 * KD * 128], F8, kind="ExternalInput")
    # sw2: col = fp*(DD*512) + d*512 + img*256 + two*128
    sw2t = nc.dram_tensor("sw2t", [128, FPS * DD * 512], F8, kind="ExternalInput")
    sb1 = nc.dram_tensor("sb1", [128, FSH], F32, kind="ExternalInput")
    yr = nc.dram_tensor("yr", [D, cpad], BF16, kind="ExternalOutput")
    ys = nc.dram_tensor("ys", [D, TS], BF16, kind="ExternalOutput")

    chunks_r = _chunk_offsets(cpad, _routed_sizes(cpad))
    xoff = [0]
    for _, s in chunks_r:
        xoff.append(xoff[-1] + KD * s)

    with tile.TileContext(nc) as tc:
        with (
            tc.tile_pool(name="wts", bufs=1) as wts,
            tc.tile_pool(name="acts", bufs=1) as acts,
            tc.tile_pool(name="hp", bufs=3) as hp,
            tc.tile_pool(name="hr", bufs=1) as hr,
            tc.tile_pool(name="op", bufs=3) as op,
            tc.tile_pool(name="ps1", bufs=2, space="PSUM") as ps1,
            tc.tile_pool(name="ps2", bufs=1, space="PSUM") as ps2,
        ):
            # ---- PE p-state warmup while the first DMAs are in flight
            # (memset on gpsimd: fast launch, no DVE spin-up dependency) ----
            warm = wts.tile([128, 512], F16, name="warm_in")
            nc.gpsimd.memset(warm[:], 0.0)
            wp = ps1.tile([128, 1024], F32, tag="p1", name="warmp")
            for _ in range(4):
                nc.tensor.matmul(wp[:, 0:512], warm[:, 0:128], warm[:],
                                 start=True, stop=True)

            # ---- resident SBUF images ----
            xs_sb = acts.tile([128, 2 * KD * TS], F8, name="xs_sb")
            sw1_sb = wts.tile([128, FSH * 2 * KD * 128], F8, name="sw1_sb")
            sw2_sb = wts.tile([128, FPS * DD * 512], F8, name="sw2_sb")
            sb1_sb = wts.tile([128, FSH], F32, name="sb1_sb")
            xg_sb = acts.tile([128, KD * cpad], F8, name="xg_sb")
            rw1_sb = wts.tile([128, FR * KD * 128], F8, name="rw1_sb")
            rw2_sb = wts.tile([128, FPR * DD * 256], F8, name="rw2_sb")
            rb1_sb = wts.tile([128, FR], F32, name="rb1_sb")
            cw_sb = acts.tile([128, cpad], F16, name="cw_sb")

            def col_dma(dst, src, lo, hi):
                nc.sync.dma_start(dst[:, lo:hi], src.ap()[:, lo:hi])

            # ---- consumption-ordered preload (HWDGE): routed chunk-0
            # first (it computes while the bigger shared stream lands),
            # then sw1/sw2 interleaved, routed L2 weights last ----
            SW1G, SW2G = 4 * 2 * KD * 128, 2 * DD * 512
            RW1G, RW2G = 4 * KD * 128, 2 * DD * 256
            col_dma(rw1_sb, rw1t, 0, RW1G)
            nc.sync.dma_start(rb1_sb[:], rb1.ap())
            col_dma(xg_sb, xg, 0, xoff[1])
            col_dma(xs_sb, xs, 0, 2 * KD * TS)
            col_dma(sw1_sb, sw1t, 0, SW1G)
            nc.sync.dma_start(sb1_sb[:], sb1.ap())
            order = [("w1", 1), ("r1", 1), ("w2", 0), ("w1", 2), ("r1", 2),
                     ("w2", 1), ("w1", 3), ("r1", 3), ("xg", 1), ("w2", 2),
                     ("w1", 4), ("w2", 3), ("w1", 5), ("xg", 2), ("w2", 4),
                     ("w1", 6), ("w2", 5), ("w1", 7), ("w2", 6), ("w2", 7)]
            for kind, g in order:
                if kind == "w1":
                    col_dma(sw1_sb, sw1t, g * SW1G, (g + 1) * SW1G)
                elif kind == "w2":
                    col_dma(sw2_sb, sw2t, g * SW2G, (g + 1) * SW2G)
                elif kind == "r1":
                    col_dma(rw1_sb, rw1t, g * RW1G, (g + 1) * RW1G)
                elif kind == "xg":
                    col_dma(xg_sb, xg, xoff[g], xoff[g + 1])
            nc.sync.dma_start(cw_sb[:], cwb.ap())
            for g in range(4):
                col_dma(rw2_sb, rw2t, g * RW2G, (g + 1) * RW2G)

            def pair3(ap2):
                """[128, 2*n] -> [128, 2, n] (DoubleRow two-k-tile view)."""
                return ap2.rearrange("p (a c) -> p a c", a=2)

            # ---- shared experts: one 512-token chunk, emitted as 16
            # f-PAIRS; L1 is a 3-term fp8-pair contraction, h is split
            # on-device into an fp8 hi/lo pair for L2's 3 terms ----
            po_s = [ps2.tile([128, TS], F32, tag=f"o{d}", name=f"po{d}")
                    for d in range(DD)]

            def psum_pair():
                # fixed 512-element slots keep both matmul outputs inside
                # one PSUM bank each, for any chunk width
                return ps1.tile([128, 1024], F32, tag="p1", name="p1")

            def act_pair(p1, cs, dst, bias_sb, f0):
                """gelu over both slots of an L1 psum pair -> dst[128,2cs]."""
                if not biased:
                    src = p1.rearrange("p (a c) -> p a c", a=2)[:, :, 0:cs]
                    d2 = dst.rearrange("p (a c) -> p a c", a=2)
                    nc.scalar.activation(d2, src, _GELU, scale=1.0 / SC)
                else:
                    for sl in range(2):
                        nc.scalar.activation(
                            dst[:, sl * cs:(sl + 1) * cs],
                            p1[:, sl * 512:sl * 512 + cs], _GELU,
                            bias=bias_sb[:, f0 + sl:f0 + sl + 1], scale=1.0 / SC)

            def sh_l1(k):
                p1 = psum_pair()
                for sl in range(2):
                    f = 2 * k + sl
                    wbase = f * (2 * KD * 128)
                    a01 = pair3(sw1_sb[:, wbase + 0:wbase + 256])
                    a23 = pair3(sw1_sb[:, wbase + 256:wbase + 512])
                    b01 = pair3(sw1_sb[:, wbase + 512:wbase + 768])
                    b23 = pair3(sw1_sb[:, wbase + 768:wbase + 1024])
                    xh01 = pair3(xs_sb[:, 0:2 * TS])
                    xh23 = pair3(xs_sb[:, 2 * TS:4 * TS])
                    xl01 = pair3(xs_sb[:, 4 * TS:6 * TS])
                    xl23 = pair3(xs_sb[:, 6 * TS:8 * TS])
                    mms = [(a01, xh01), (a01, xl01), (a23, xh23), (a23, xl23),
                           (b01, xh01), (b23, xh23)]
                    out = p1[:, sl * 512:(sl + 1) * 512]
                    for i, (w, x) in enumerate(mms):
                        nc.tensor.matmul(out, w, x, start=(i == 0),
                                         stop=(i == len(mms) - 1), perf_mode=DR)
                h16 = hp.tile([128, 2 * TS], F16, name="h16")
                hh = hp.tile([128, 2 * TS], F8, name="hh")
                hl = hp.tile([128, 2 * TS], F8, name="hl")
                act_pair(p1, TS, h16, sb1_sb, 2 * k)
                nc.vector.tensor_copy(hh[:], h16[:])
                nc.gpsimd.tensor_sub(hl[:], h16[:], hh[:])
                return (hh, hl)

            def sh_l2(fp, hpair):
                hh2, hl2 = pair3(hpair[0][:]), pair3(hpair[1][:])
                for d in range(DD):
                    base = fp * (DD * 512) + d * 512
                    a2 = pair3(sw2_sb[:, base:base + 256])
                    b2 = pair3(sw2_sb[:, base + 256:base + 512])
                    out = po_s[d][:]
                    nc.tensor.matmul(out, a2, hh2, start=(fp == 0), stop=False,
                                     perf_mode=DR)
                    nc.tensor.matmul(out, a2, hl2, start=False, stop=False,
                                     perf_mode=DR)
                    nc.tensor.matmul(out, b2, hh2, start=False,
                                     stop=(fp == FPS - 1), perf_mode=DR)

            def sh_drain():
                o = op.tile([128, DD * TS], BF16, name="o_s")
                for d in range(DD):
                    nc.vector.tensor_scalar_mul(
                        o[:, d * TS:(d + 1) * TS], po_s[d][:], 1.0 / SC)
                ydst = ys.ap().rearrange("(dd p) c -> p dd c", p=128)
                nc.gpsimd.dma_start(ydst, o.rearrange("p (dd c) -> p dd c", dd=DD))

            # ---- routed expert: L1+GELU interleaves into the shared
            # phase (ACT has slack there); every h8 pair persists in SBUF
            # (~17.5KB/partition) so L2 can run as a pure-PE tail ----
            h8_tiles: dict = {}

            def ro_l1(ci, j):
                c0, cs = chunks_r[ci]
                p1 = psum_pair()
                for sl in range(2):
                    f = 2 * j + sl
                    wbase = f * (KD * 128)
                    w01 = pair3(rw1_sb[:, wbase + 0:wbase + 256])
                    w23 = pair3(rw1_sb[:, wbase + 256:wbase + 512])
                    xb = xoff[ci]
                    x01 = pair3(xg_sb[:, xb + 0:xb + 2 * cs])
                    x23 = pair3(xg_sb[:, xb + 2 * cs:xb + 4 * cs])
                    out = p1[:, sl * 512:sl * 512 + cs]
                    nc.tensor.matmul(out, w01, x01, start=True, stop=False,
                                     perf_mode=DR)
                    nc.tensor.matmul(out, w23, x23, start=False, stop=True,
                                     perf_mode=DR)
                h8 = hr.tile([128, 2 * cs], F8, name=f"h8_{ci}_{j}")
                act_pair(p1, cs, h8, rb1_sb, 2 * j)
                h8_tiles[(ci, j)] = h8

            def ro_l2(ci, fp, po):
                cs = chunks_r[ci][1]
                h2 = pair3(h8_tiles[(ci, fp)][:])
                for d in range(DD):
                    base = fp * (DD * 256) + d * 256
                    w2 = pair3(rw2_sb[:, base:base + 256])
                    nc.tensor.matmul(po[d][:], w2, h2,
                                     start=(fp == 0), stop=(fp == FPR - 1),
                                     perf_mode=DR)

            def ro_drain(ci, po):
                c0, cs = chunks_r[ci]
                o = op.tile([128, DD * cs], BF16, name="o_r")
                for d in range(DD):
                    nc.vector.tensor_mul(o[:, d * cs:(d + 1) * cs], po[d][:],
                                         cw_sb[:, c0:c0 + cs])
                ydst = yr.ap().rearrange("(dd p) c -> p dd c", p=128)[:, :, c0:c0 + cs]
                ysrc = o.rearrange("p (dd c) -> p dd c", dd=DD)
                if ci == len(chunks_r) - 1:
                    nc.sync.dma_start(ydst, ysrc)
                else:
                    nc.gpsimd.dma_start(ydst, ysrc)

            # ---- interleaved emission over PAIR steps: a few leading
            # routed pairs cover the DMA ramp, then shared/routed pairs
            # alternate; shared L2 lags its pair by LA steps ----
            rps = [(ci, j) for ci in range(len(chunks_r)) for j in range(FPR)]
            units: list = [("r", *rps[i]) for i in range(6)]
            ri = 6
            for k in range(FPS):
                units.append(("s", k))
                if ri < len(rps):
                    units.append(("r", *rps[ri]))
                    ri += 1
            units += [("r", *rps[i]) for i in range(ri, len(rps))]
            LA = 2
            pend: list = []   # head-blocking FIFO: pops in append order

            for i in range(len(units) + LA + 2):
                if i < len(units):
                    u = units[i]
                    if u[0] == "s":
                        hpair = sh_l1(u[1])
                        pend.append((i + LA, "s", (u[1], hpair)))
                        if u[1] == FPS - 1:
                            pend.append((i + LA + 1, "sd", None))
                    else:
                        ro_l1(u[1], u[2])
                while pend and pend[0][0] <= i:
                    _, kind, pl = pend.pop(0)
                    if kind == "s":
                        sh_l2(*pl)
                    else:
                        sh_drain()

            # ---- routed L2: pure-PE tail (no ACT dependency left) ----
            for ci in range(len(chunks_r)):
                cs = chunks_r[ci][1]
                po = [ps2.tile([128, cs], F32, tag=f"o{d}", name=f"po{d}")
                      for d in range(DD)]
                for fp in range(FPR):
                    ro_l2(ci, fp, po)
                ro_drain(ci, po)

    nc.compile()
    return nc


def _f8(a):
    return np.asarray(a, np_f8)


def _hilo(a):
    """Same-scale fp8 pair: hi = fp8(a), lo = fp8(a - hi)."""
    hi = _f8(a)
    lo = _f8(np.asarray(a, np.float32) - hi.astype(np.float32))
    return hi, lo


def _pack_sw1(sw1):
    W1 = sw1.reshape(FS, D).astype(np.float32) * SC
    hi, lo = _hilo(W1)
    st = np.stack([hi, lo])                         # [img, FS, D]
    v = st.reshape(2, FSH, 128, KD, 128)            # img f ff k kk
    v = v.transpose(4, 1, 0, 3, 2)                  # kk f img k ff
    return np.ascontiguousarray(v.reshape(128, -1))


def _pack_sw2(sw2):
    W2 = sw2.transpose(0, 2, 1).reshape(FS, D).astype(np.float32) * SC
    hi, lo = _hilo(W2)
    st = np.stack([hi, lo])                         # [img, FS, D]
    v = st.reshape(2, FPS, 2, 128, DD, 128)         # img fp two ff d dd
    v = v.transpose(3, 1, 4, 0, 2, 5)               # ff fp d img two dd
    return np.ascontiguousarray(v.reshape(128, -1))


def _pack_rw1(w):
    A = _f8(w.astype(np.float32) * SC)              # [F, D]
    v = A.reshape(FR, 128, KD, 128)                 # f ff k kk
    v = v.transpose(3, 0, 2, 1)                     # kk f k ff
    return np.ascontiguousarray(v.reshape(128, -1))


def _pack_rw2(w):
    A = _f8(w.T.astype(np.float32) * SC)            # [F, D]
    v = A.reshape(FPR, 2, 128, DD, 128)             # fp two ff d dd
    v = v.transpose(2, 0, 3, 1, 4)                  # ff fp d two dd
    return np.ascontiguousarray(v.reshape(128, -1))


def _pack_xs(xT):
    hi, lo = _hilo(xT)                              # [D, TS] each
    st = np.stack([hi, lo])                         # img D TS
    v = st.reshape(2, KD, 128, TS).transpose(2, 0, 1, 3)   # kk img k c
    return np.ascontiguousarray(v.reshape(128, -1))


def _pack_xg(xT, sizes):
    parts = []
    for c0, cs in _chunk_offsets(xT.shape[1], sizes):
        blk = xT[:, c0:c0 + cs]
        parts.append(blk.reshape(KD, 128, cs).transpose(1, 0, 2).reshape(128, -1))
    return np.ascontiguousarray(np.concatenate(parts, axis=1))


def kernel(x, gate_w, gate_b, sw1, sb1, sw2, sb2, rw1, rb1, rw2, rb2):
    x = np.asarray(x, np.float32)
    gate_w = np.asarray(gate_w, np.float32)
    gate_b = np.asarray(gate_b, np.float32)
    sw1 = np.asarray(sw1, np.float32)
    sb1 = np.asarray(sb1, np.float32)
    sw2 = np.asarray(sw2, np.float32)
    sb2 = np.asarray(sb2, np.float32)
    rw1 = np.asarray(rw1, np.float32)
    rb1 = np.asarray(rb1, np.float32)
    rw2 = np.asarray(rw2, np.float32)
    rb2 = np.asarray(rb2, np.float32)

    t = x.reshape(T, D)

    # ---- router on host (part of the dispatch/sharding step) ----
    logits = t @ gate_w.T + gate_b
    m = logits.max(axis=1, keepdims=True)
    ex = np.exp(logits - m)
    probs = ex / ex.sum(axis=1, keepdims=True)
    top_i = np.argpartition(-probs, K - 1, axis=1)[:, :K]          # [T, K]

    sel = np.zeros((T, E), bool)
    sel[np.arange(T)[:, None], top_i] = True
    idxs = [np.nonzero(sel[:, e])[0] for e in range(E)]
    counts = np.array([len(i) for i in idxs])
    cpad = max(CHUNK, int(-(-counts.max() // 4) * 4))

    biased = bool(rb1.any() or sb1.any())
    key = (cpad, biased)
    if key not in _cache:
        _cache[key] = _build(cpad, biased)
    nc = _cache[key]

    sw1t = _pack_sw1(sw1)
    sw2t = _pack_sw2(sw2)
    sb1c = np.ascontiguousarray(sb1.reshape(FSH, 128).T)
    rsizes = _routed_sizes(cpad)

    in_maps = []
    for c in range(N_CORES):
        idx = idxs[c]
        ce = len(idx)
        xgT = np.zeros((D, cpad), np_f8)
        xgT[:, :ce] = _f8(t[idx].T)
        cwbm = np.zeros((128, cpad), np.float16)
        cwbm[:, :ce] = (probs[idx, c] / SC)[None, :].astype(np.float16)
        in_maps.append({
            "xg": _pack_xg(xgT, rsizes),
            "cwb": cwbm,
            "rw1t": _pack_rw1(rw1[c]),
            "rw2t": _pack_rw2(rw2[c]),
            "rb1": np.ascontiguousarray(rb1[c].reshape(FR, 128).T),
            "xs": _pack_xs(np.ascontiguousarray(t[c * TS:(c + 1) * TS].T)),
            "sw1t": sw1t,
            "sw2t": sw2t,
            "sb1": sb1c,
        })

    res = run_bass_kernel_spmd(nc, in_maps, core_ids=list(range(N_CORES)))

    # ---- combine on host ----
    out = np.empty((T, D), np.float32)
    for c in range(N_CORES):
        out[c * TS:(c + 1) * TS] = res.results[c]["ys"].T.astype(np.float32)
    for c in range(N_CORES):
        idx = idxs[c]
        out[idx] += res.results[c]["yr"][:, :len(idx)].T.astype(np.float32)

    # output biases (zero in the spec, handled exactly anyway)
    if sb2.any() or rb2.any():
        cw = np.zeros((T, E), np.float32)
        np.add.at(cw, (np.arange(T)[:, None], top_i),
                  np.take_along_axis(probs, top_i, axis=1))
        out += sb2.sum(axis=0)[None, :] + cw @ rb2

    return out.reshape(B, S, D)


# revision 19
# speedup vs baseline: 1.0603x; 1.0603x over previous
"""DeepseekMoE kernel for 8 Trainium2 NeuronCores.

Strategy (expert-parallel routed + data-parallel shared, fp8 PE path):
  - Host computes the router (gate matmul, softmax, top-2) in numpy and
    gathers each expert's tokens (classic MoE dispatch, done host-side as
    part of sharding).  Core c runs routed expert c's FFN over its
    gathered tokens (padded to a common cpad so all 8 cores run the same
    SPMD program); shared experts are replicated and each core runs them
    over a distinct 512-token slice of the batch.
  - All matmuls run in fp8-E4M3 with the PE's DoubleRow perf mode (two
    128-row k-tiles contracted per instruction at 0.5 cycles/row -> 4x
    the fp16 rate).  Weights are scaled by 64 before quantization so
    their 0.02-scale values leave the E4M3 subnormal range; the 1/64
    descale folds into the GELU's input scale (layer 1), the combine
    weights (routed layer 2) or the output copy (shared layer 2).
  - The routed path (only ~4.5% of the output energy after the top-2
    combine weights) tolerates plain fp8 quantization of x, h and both
    weight matrices (~1.1e-2 end-to-end rel err vs the 2e-2 budget).
  - The shared path (~97% of output energy) cannot; each shared operand
    is a same-scale fp8 PAIR (hi = fp8(v), lo = fp8(v - hi)) and each
    matmul contracts 3 of the 4 cross terms -- hi@a + lo@a + hi@b --
    which restores ~fp16 accuracy at 0.75x the fp16 PE cost.  x's pair
    is built on the host; h's pair is built on-device (ACT writes
    gelu->fp16, DVE casts the fp8 hi image, Pool subtracts for lo).
  - Shared phase runs FIRST: its 8 MB of weight DMA streams in
    consumption order behind a 41 us compute phase, and the (smaller)
    routed weights stream during the shared tail, so the DMA engines are
    never on the critical path.
  - Layout is fully transposed (features on partitions, tokens on the
    free dim) so the two FFN layers chain with no on-chip transposes;
    every operand is host-packed so each DMA is a contiguous >=2KB-per-
    partition transfer.  Outputs ride the SWDGE (gpsimd) path except the
    final chunk (HWDGE, shorter tail).
  - Host scatters per-expert outputs back (each token appears in K=2
    experts) and adds the (zero, but handled exactly) output biases.
"""

import numpy as np
import ml_dtypes

import concourse.bass as bass
import concourse.tile as tile
import concourse.mybir as mybir
from concourse import bacc
from concourse.bass_utils import run_bass_kernel_spmd

B, S, D, F, E, NS, K = 2, 2048, 512, 2048, 8, 2, 2
T = B * S
N_CORES = 8
TS = T // N_CORES          # shared-expert tokens per core
FS = NS * F                # concatenated shared FFN width (4096)
CHUNK = 512                # token chunk (= max fp32 PSUM bank free dim)
KD = D // 128              # 4  k-tiles over D
FR = F // 128              # 16 f-tiles routed
FPR = FR // 2              # 8  routed f-pairs
FSH = FS // 128            # 32 f-tiles shared
FPS = FSH // 2             # 16 shared f-pairs
DD = D // 128              # 4  output d-tiles
SC = 64.0                  # fp8 weight pre-scale (lifts 0.02-scale
                           # weights out of E4M3's subnormal range)

F8 = mybir.dt.float8e4
F16 = mybir.dt.float16
BF16 = mybir.dt.bfloat16
F32 = mybir.dt.float32
np_f8 = ml_dtypes.float8_e4m3
np_bf16 = ml_dtypes.bfloat16

DR = mybir.MatmulPerfMode.DoubleRow
_GELU = mybir.ActivationFunctionType.Gelu

_cache: dict = {}


def _routed_sizes(cpad):
    """Token-chunk sizes for the routed phase: 512s in the middle and a
    128-wide final chunk so the end-of-kernel drain (combine + DMA) is
    as short as possible."""
    if cpad <= 2 * 128:
        return [cpad]
    rem = cpad - 128
    sizes = []
    while rem > CHUNK:
        sizes.append(CHUNK)
        rem -= CHUNK
    return [rem] + sizes + [128]


def _chunk_offsets(total, sizes):
    out, c0 = [], 0
    for s in sizes:
        out.append((c0, s))
        c0 += s
    return out


def _build(cpad: int, biased: bool = False):
    nc = bacc.Bacc("TRN2", debug=False)

    # -- routed inputs (expert c on core c), all fp8 pre-scaled by 64 --
    xg = nc.dram_tensor("xg", [128, KD * cpad], F8, kind="ExternalInput")
    cwb = nc.dram_tensor("cwb", [128, cpad], F16, kind="ExternalInput")
    # rw1: col = f*(KD*128) + k*128 ; stationary [k-rows, f-cols] blocks
    rw1t = nc.dram_tensor("rw1t", [128, FR * KD * 128], F8, kind="ExternalInput")
    # rw2: col = fp*(DD*256) + d*256 + two*128
    rw2t = nc.dram_tensor("rw2t", [128, FPR * DD * 256], F8, kind="ExternalInput")
    rb1 = nc.dram_tensor("rb1", [128, FR], F32, kind="ExternalInput")
    # -- shared inputs, fp8 hi/lo pairs --
    # xs: col = img*(KD*TS) + k*TS   (img 0 = hi, 1 = lo)
    xs = nc.dram_tensor("xs", [128, 2 * KD * TS], F8, kind="ExternalInput")
    # sw1: col = f*(2*KD*128) + img*(KD*128) + k*128
    sw1t = nc.dram_tensor("sw1t", [128, FSH * 2 * KD * 128], F8, kind="ExternalInput")
    # sw2: col = fp*(DD*512) + d*512 + img*256 + two*128
    sw2t = nc.dram_tensor("sw2t", [128, FPS * DD * 512], F8, kind="ExternalInput")
    sb1 = nc.dram_tensor("sb1", [128, FSH], F32, kind="ExternalInput")
    yr = nc.dram_tensor("yr", [D, cpad], BF16, kind="ExternalOutput")
    ys = nc.dram_tensor("ys", [D, TS], BF16, kind="ExternalOutput")

    chunks_r = _chunk_offsets(cpad, _routed_sizes(cpad))
    xoff = [0]
    for _, s in chunks_r:
        xoff.append(xoff[-1] + KD * s)

    with tile.TileContext(nc) as tc:
        with (
            tc.tile_pool(name="wts", bufs=1) as wts,
            tc.tile_pool(name="acts", bufs=1) as acts,
            tc.tile_pool(name="hp16", bufs=3) as hp16,
            tc.tile_pool(name="hp8", bufs=6) as hp8,
            tc.tile_pool(name="hr", bufs=1) as hr,
            tc.tile_pool(name="op", bufs=3) as op,
            tc.tile_pool(name="ps1", bufs=2, space="PSUM") as ps1,
            tc.tile_pool(name="ps2", bufs=1, space="PSUM") as ps2,
        ):
            # ---- PE p-state warmup while the first DMAs are in flight ----
            warm = wts.tile([128, 512], F16, name="warm_in")
            nc.vector.memset(warm[:], 0.0)
            wp = ps1.tile([128, 1024], F32, tag="p1", name="warmp")
            for _ in range(6):
                nc.tensor.matmul(wp[:, 0:512], warm[:, 0:128], warm[:],
                                 start=True, stop=True)

            # ---- resident SBUF images ----
            xs_sb = acts.tile([128, 2 * KD * TS], F8, name="xs_sb")
            sw1_sb = wts.tile([128, FSH * 2 * KD * 128], F8, name="sw1_sb")
            sw2_sb = wts.tile([128, FPS * DD * 512], F8, name="sw2_sb")
            sb1_sb = wts.tile([128, FSH], F32, name="sb1_sb")
            xg_sb = acts.tile([128, KD * cpad], F8, name="xg_sb")
            rw1_sb = wts.tile([128, FR * KD * 128], F8, name="rw1_sb")
            rw2_sb = wts.tile([128, FPR * DD * 256], F8, name="rw2_sb")
            rb1_sb = wts.tile([128, FR], F32, name="rb1_sb")
            cw_sb = acts.tile([128, cpad], F16, name="cw_sb")

            # ---- consumption-ordered preload on TWO parallel queues.
            # Each DMA instruction costs ~1.5us of queue occupancy on top
            # of its transfer, so transfers are merged into ~8KB/partition
            # groups and split across the SP (HWDGE) and Pool (SWDGE)
            # rings: SP carries the sw1 stream, Pool carries everything
            # the routed phase needs plus the sw2 stream. ----
            SW1G, SW2G = 8 * 2 * KD * 128, 4 * DD * 512
            RW1G = 8 * KD * 128

            def col_dma(q, dst, src, lo, hi):
                q.dma_start(dst[:, lo:hi], src.ap()[:, lo:hi])

            sp, pl = nc.sync, nc.gpsimd
            # SP: shared L1 stream (consumed from the first pair-step on)
            col_dma(sp, sw1_sb, sw1t, 0, SW1G)
            sp.dma_start(sb1_sb[:], sb1.ap())
            for g in range(1, 4):
                col_dma(sp, sw1_sb, sw1t, g * SW1G, (g + 1) * SW1G)
            sp.dma_start(rb1_sb[:], rb1.ap())
            col_dma(sp, xg_sb, xg, xoff[1], xoff[-1])     # later routed chunks
            # Pool: xs + routed chunk-0 + sw2 stream + routed L2 weights
            pl.dma_start(xs_sb[:], xs.ap())
            col_dma(pl, rw1_sb, rw1t, 0, RW1G)
            col_dma(pl, xg_sb, xg, 0, xoff[1])
            col_dma(pl, rw1_sb, rw1t, RW1G, 2 * RW1G)
            for g in range(4):
                col_dma(pl, sw2_sb, sw2t, g * SW2G, (g + 1) * SW2G)
            pl.dma_start(cw_sb[:], cwb.ap())
            col_dma(pl, rw2_sb, rw2t, 0, FPR * DD * 256)

            def pair3(ap2):
                """[128, 2*n] -> [128, 2, n] (DoubleRow two-k-tile view)."""
                return ap2.rearrange("p (a c) -> p a c", a=2)

            # ---- shared experts: one 512-token chunk, emitted as 16
            # f-PAIRS; L1 is a 3-term fp8-pair contraction, h is split
            # on-device into an fp8 hi/lo pair for L2's 3 terms ----
            po_s = [ps2.tile([128, TS], F32, tag=f"o{d}", name=f"po{d}")
                    for d in range(DD)]

            def psum_pair():
                # fixed 512-element slots keep both matmul outputs inside
                # one PSUM bank each, for any chunk width
                return ps1.tile([128, 1024], F32, tag="p1", name="p1")

            def act_pair(p1, cs, dst, bias_sb, f0):
                """gelu over both slots of an L1 psum pair -> dst[128,2cs]."""
                if not biased:
                    src = p1.rearrange("p (a c) -> p a c", a=2)[:, :, 0:cs]
                    d2 = dst.rearrange("p (a c) -> p a c", a=2)
                    nc.scalar.activation(d2, src, _GELU, scale=1.0 / SC)
                else:
                    for sl in range(2):
                        nc.scalar.activation(
                            dst[:, sl * cs:(sl + 1) * cs],
                            p1[:, sl * 512:sl * 512 + cs], _GELU,
                            bias=bias_sb[:, f0 + sl:f0 + sl + 1], scale=1.0 / SC)

            def sh_l1(k):
                p1 = psum_pair()
                for sl in range(2):
                    f = 2 * k + sl
                    wbase = f * (2 * KD * 128)
                    a01 = pair3(sw1_sb[:, wbase + 0:wbase + 256])
                    a23 = pair3(sw1_sb[:, wbase + 256:wbase + 512])
                    b01 = pair3(sw1_sb[:, wbase + 512:wbase + 768])
                    b23 = pair3(sw1_sb[:, wbase + 768:wbase + 1024])
                    xh01 = pair3(xs_sb[:, 0:2 * TS])
                    xh23 = pair3(xs_sb[:, 2 * TS:4 * TS])
                    xl01 = pair3(xs_sb[:, 4 * TS:6 * TS])
                    xl23 = pair3(xs_sb[:, 6 * TS:8 * TS])
                    mms = [(a01, xh01), (a01, xl01), (a23, xh23), (a23, xl23),
                           (b01, xh01), (b23, xh23)]
                    out = p1[:, sl * 512:(sl + 1) * 512]
                    for i, (w, x) in enumerate(mms):
                        nc.tensor.matmul(out, w, x, start=(i == 0),
                                         stop=(i == len(mms) - 1), perf_mode=DR)
                h16 = hp16.tile([128, 2 * TS], F16, name="h16")
                hh = hp8.tile([128, 2 * TS], F8, name="hh")
                hl = hp8.tile([128, 2 * TS], F8, name="hl")
                act_pair(p1, TS, h16, sb1_sb, 2 * k)
                nc.vector.tensor_copy(hh[:], h16[:])
                nc.vector.tensor_sub(hl[:], h16[:], hh[:])
                return (hh, hl)

            def sh_l2(fp, hpair):
                hh2, hl2 = pair3(hpair[0][:]), pair3(hpair[1][:])
                for d in range(DD):
                    base = fp * (DD * 512) + d * 512
                    a2 = pair3(sw2_sb[:, base:base + 256])
                    b2 = pair3(sw2_sb[:, base + 256:base + 512])
                    out = po_s[d][:]
                    nc.tensor.matmul(out, a2, hh2, start=(fp == 0), stop=False,
                                     perf_mode=DR)
                    nc.tensor.matmul(out, a2, hl2, start=False, stop=False,
                                     perf_mode=DR)
                    nc.tensor.matmul(out, b2, hh2, start=False,
                                     stop=(fp == FPS - 1), perf_mode=DR)

            def sh_drain():
                o = op.tile([128, DD * TS], BF16, name="o_s")
                for d in range(DD):
                    nc.vector.tensor_scalar_mul(
                        o[:, d * TS:(d + 1) * TS], po_s[d][:], 1.0 / SC)
                ydst = ys.ap().rearrange("(dd p) c -> p dd c", p=128)
                nc.gpsimd.dma_start(ydst, o.rearrange("p (dd c) -> p dd c", dd=DD))

            # ---- routed expert: L1+GELU interleaves into the shared
            # phase (ACT has slack there); every h8 pair persists in SBUF
            # (~17.5KB/partition) so L2 can run as a pure-PE tail ----
            h8_tiles: dict = {}

            def ro_l1(ci, j):
                c0, cs = chunks_r[ci]
                p1 = psum_pair()
                for sl in range(2):
                    f = 2 * j + sl
                    wbase = f * (KD * 128)
                    w01 = pair3(rw1_sb[:, wbase + 0:wbase + 256])
                    w23 = pair3(rw1_sb[:, wbase + 256:wbase + 512])
                    xb = xoff[ci]
                    x01 = pair3(xg_sb[:, xb + 0:xb + 2 * cs])
                    x23 = pair3(xg_sb[:, xb + 2 * cs:xb + 4 * cs])
                    out = p1[:, sl * 512:sl * 512 + cs]
                    nc.tensor.matmul(out, w01, x01, start=True, stop=False,
                                     perf_mode=DR)
                    nc.tensor.matmul(out, w23, x23, start=False, stop=True,
                                     perf_mode=DR)
                h8 = hr.tile([128, 2 * cs], F8, name=f"h8_{ci}_{j}")
                act_pair(p1, cs, h8, rb1_sb, 2 * j)
                h8_tiles[(ci, j)] = h8

            def ro_l2(ci, fp, po):
                cs = chunks_r[ci][1]
                h2 = pair3(h8_tiles[(ci, fp)][:])
                for d in range(DD):
                    base = fp * (DD * 256) + d * 256
                    w2 = pair3(rw2_sb[:, base:base + 256])
                    nc.tensor.matmul(po[d][:], w2, h2,
                                     start=(fp == 0), stop=(fp == FPR - 1),
                                     perf_mode=DR)

            def ro_drain(ci, po):
                c0, cs = chunks_r[ci]
                o = op.tile([128, DD * cs], BF16, name="o_r")
                for d in range(DD):
                    nc.vector.tensor_mul(o[:, d * cs:(d + 1) * cs], po[d][:],
                                         cw_sb[:, c0:c0 + cs])
                ydst = yr.ap().rearrange("(dd p) c -> p dd c", p=128)[:, :, c0:c0 + cs]
                ysrc = o.rearrange("p (dd c) -> p dd c", dd=DD)
                if ci == len(chunks_r) - 1:
                    nc.sync.dma_start(ydst, ysrc)
                else:
                    nc.gpsimd.dma_start(ydst, ysrc)

            # ---- interleaved emission over PAIR steps: two shared pairs
            # lead (their weights arrive first), then routed pairs mix in
            # at a ~1.5:1 ratio.  Shared L2 lags its pair by LA steps and
            # is additionally deferred until the sw2 stream has landed ----
            rps = [(ci, j) for ci in range(len(chunks_r)) for j in range(FPR)]
            units: list = [("s", 0), ("s", 1)]
            ri = 0
            for k in range(2, FPS):
                units.append(("r", *rps[ri]))
                ri += 1
                units.append(("s", k))
                if k % 2 == 0 and ri < len(rps):
                    units.append(("r", *rps[ri]))
                    ri += 1
            units += [("r", *rps[i]) for i in range(ri, len(rps))]
            LA = 3
            FIRST_L2 = 10     # defer L2 so early Pool-queue DMAs (routed
                              # x/weights) beat the sw2 stream
            pend: list = []   # head-blocking FIFO: pops in append order

            for i in range(len(units) + LA + 2):
                if i < len(units):
                    u = units[i]
                    if u[0] == "s":
                        k = u[1]
                        hpair = sh_l1(k)
                        pend.append((max(i + LA, FIRST_L2 + k), "s", (k, hpair)))
                        if k == FPS - 1:
                            pend.append((max(i + LA, FIRST_L2 + FPS) + 1,
                                         "sd", None))
                    else:
                        ro_l1(u[1], u[2])
                while pend and pend[0][0] <= i:
                    _, kind, pl = pend.pop(0)
                    if kind == "s":
                        sh_l2(*pl)
                    else:
                        sh_drain()

            # ---- routed L2: pure-PE tail (no ACT dependency left) ----
            for ci in range(len(chunks_r)):
                cs = chunks_r[ci][1]
                po = [ps2.tile([128, cs], F32, tag=f"o{d}", name=f"po{d}")
                      for d in range(DD)]
                for fp in range(FPR):
                    ro_l2(ci, fp, po)
                ro_drain(ci, po)

    nc.compile()
    return nc


def _f8(a):
    return np.asarray(a, np_f8)


def _hilo(a):
    """Same-scale fp8 pair: hi = fp8(a), lo = fp8(a - hi)."""
    hi = _f8(a)
    lo = _f8(np.asarray(a, np.float32) - hi.astype(np.float32))
    return hi, lo


def _pack_sw1(sw1):
    W1 = sw1.reshape(FS, D).astype(np.float32) * SC
    hi, lo = _hilo(W1)
    st = np.stack([hi, lo])                         # [img, FS, D]
    v = st.reshape(2, FSH, 128, KD, 128)            # img f ff k kk
    v = v.transpose(4, 1, 0, 3, 2)                  # kk f img k ff
    return np.ascontiguousarray(v.reshape(128, -1))


def _pack_sw2(sw2):
    W2 = sw2.transpose(0, 2, 1).reshape(FS, D).astype(np.float32) * SC
    hi, lo = _hilo(W2)
    st = np.stack([hi, lo])                         # [img, FS, D]
    v = st.reshape(2, FPS, 2, 128, DD, 128)         # img fp two ff d dd
    v = v.transpose(3, 1, 4, 0, 2, 5)               # ff fp d img two dd
    return np.ascontiguousarray(v.reshape(128, -1))


def _pack_rw1(w):
    A = _f8(w.astype(np.float32) * SC)              # [F, D]
    v = A.reshape(FR, 128, KD, 128)                 # f ff k kk
    v = v.transpose(3, 0, 2, 1)                     # kk f k ff
    return np.ascontiguousarray(v.reshape(128, -1))


def _pack_rw2(w):
    A = _f8(w.T.astype(np.float32) * SC)            # [F, D]
    v = A.reshape(FPR, 2, 128, DD, 128)             # fp two ff d dd
    v = v.transpose(2, 0, 3, 1, 4)                  # ff fp d two dd
    return np.ascontiguousarray(v.reshape(128, -1))


def _pack_xs(xT):
    hi, lo = _hilo(xT)                              # [D, TS] each
    st = np.stack([hi, lo])                         # img D TS
    v = st.reshape(2, KD, 128, TS).transpose(2, 0, 1, 3)   # kk img k c
    return np.ascontiguousarray(v.reshape(128, -1))


def _pack_xg(xT, sizes):
    parts = []
    for c0, cs in _chunk_offsets(xT.shape[1], sizes):
        blk = xT[:, c0:c0 + cs]
        parts.append(blk.reshape(KD, 128, cs).transpose(1, 0, 2).reshape(128, -1))
    return np.ascontiguousarray(np.concatenate(parts, axis=1))


def kernel(x, gate_w, gate_b, sw1, sb1, sw2, sb2, rw1, rb1, rw2, rb2):
    x = np.asarray(x, np.float32)
    gate_w = np.asarray(gate_w, np.float32)
    gate_b = np.asarray(gate_b, np.float32)
    sw1 = np.asarray(sw1, np.float32)
    sb1 = np.asarray(sb1, np.float32)
    sw2 = np.asarray(sw2, np.float32)
    sb2 = np.asarray(sb2, np.float32)
    rw1 = np.asarray(rw1, np.float32)
    rb1 = np.asarray(rb1, np.float32)
    rw2 = np.asarray(rw2, np.float32)
    rb2 = np.asarray(rb2, np.float32)

    t = x.reshape(T, D)

    # ---- router on host (part of the dispatch/sharding step) ----
    logits = t @ gate_w.T + gate_b
    m = logits.max(axis=1, keepdims=True)
    ex = np.exp(logits - m)
    probs = ex / ex.sum(axis=1, keepdims=True)
    top_i = np.argpartition(-probs, K - 1, axis=1)[:, :K]          # [T, K]

    sel = np.zeros((T, E), bool)
    sel[np.arange(T)[:, None], top_i] = True
    idxs = [np.nonzero(sel[:, e])[0] for e in range(E)]
    counts = np.array([len(i) for i in idxs])
    cpad = max(CHUNK, int(-(-counts.max() // 4) * 4))

    biased = bool(rb1.any() or sb1.any())
    key = (cpad, biased)
    if key not in _cache:
        _cache[key] = _build(cpad, biased)
    nc = _cache[key]

    sw1t = _pack_sw1(sw1)
    sw2t = _pack_sw2(sw2)
    sb1c = np.ascontiguousarray(sb1.reshape(FSH, 128).T)
    rsizes = _routed_sizes(cpad)

    in_maps = []
    for c in range(N_CORES):
        idx = idxs[c]
        ce = len(idx)
        xgT = np.zeros((D, cpad), np_f8)
        xgT[:, :ce] = _f8(t[idx].T)
        cwbm = np.zeros((128, cpad), np.float16)
        cwbm[:, :ce] = (probs[idx, c] / SC)[None, :].astype(np.float16)
        in_maps.append({
            "xg": _pack_xg(xgT, rsizes),
            "cwb": cwbm,
            "rw1t": _pack_rw1(rw1[c]),
            "rw2t": _pack_rw2(rw2[c]),
            "rb1": np.ascontiguousarray(rb1[c].reshape(FR, 128).T),
            "xs": _pack_xs(np.ascontiguousarray(t[c * TS:(c + 1) * TS].T)),
            "sw1t": sw1t,
            "sw2t": sw2t,
            "sb1": sb1c,
        })

    res = run_bass_kernel_spmd(nc, in_maps, core_ids=list(range(N_CORES)))

    # ---- combine on host ----
    out = np.empty((T, D), np.float32)
    for c in range(N_CORES):
        out[c * TS:(c + 1) * TS] = res.results[c]["ys"].T.astype(np.float32)
    for c in range(N_CORES):
        idx = idxs[c]
        out[idx] += res.results[c]["yr"][:, :len(idx)].T.astype(np.float32)

    # output biases (zero in the spec, handled exactly anyway)
    if sb2.any() or rb2.any():
        cw = np.zeros((T, E), np.float32)
        np.add.at(cw, (np.arange(T)[:, None], top_i),
                  np.take_along_axis(probs, top_i, axis=1))
        out += sb2.sum(axis=0)[None, :] + cw @ rb2

    return out.reshape(B, S, D)
